# revision 1
# baseline (speedup 1.0000x reference)
# Multi-head attention block (B=4, N=2048, DIM=512, H=8, HD=64) on 8 TRN2 cores.
#
# Sharding: core c handles batch b = c//2 and a 4-head group hg = c%2
# (heads 4*hg .. 4*hg+3).  Each core computes a partial output-projection
# part_c = ctx_hg @ Wo[hg] of shape [N, DIM]; the host sums the two partials
# per batch.
#
# Device dataflow (activations kept transposed, [feature, position]):
#   xT    [512, 2048]   host-transposed input slice (f32r)
#   qdup_h [128, 2048]  q rows for head h duplicated on both partition halves
#   kdiag_h [128,16,128] block-diagonal stationary: diag(kT_h[jcA], kT_h[jcB])
#                       so the QK^T matmul runs with K=128 (K=64 is ~1.6x
#                       slower on HW)
#   v_aug [j, 65] per head, col 64 = 1.0: the PV matmul's row 64 gives the
#                       softmax denominator for free
#   simT  [j,i] per head -> exp on ACT ([128,1024] ops to amortize the ~352cy
#                       ACTIVATE startup) -> ctxT[65,i] accumulation
#   ctxT normalized by a DMA-bounce-broadcast reciprocal of row 64, then
#   part [n, 512] = ctxT^T-stationary @ Wo_s.
#
# The attention loop is software-pipelined BY EMISSION ORDER: sim for step
# j+1 is emitted before PV of step j, so the in-order PE stream never stalls
# on ACT's exp.
import os
import numpy as np

B, N, DIM = 4, 2048, 512
HEADS, HD = 8, 64
HG = 2                      # head groups (cores per batch)
MH = HEADS // HG            # heads per core = 4
M = MH * HD                 # per-core hidden slice = 256
P = 128
KC = DIM // P               # 4 contraction chunks for projections
MC = M // P                 # 2 m-chunks
IB = 1024                   # exp/i-block width
NB = N // IB                # 2 i-blocks
NCH = N // P                # 16 j chunks
SCALE = HD ** -0.5

_CACHE = {}


def _dts():
    from concourse import mybir
    sim = os.environ.get("BASSK_SIMDT", "bf16")
    e = os.environ.get("BASSK_EDT", "f32r")
    m = {"bf16": mybir.dt.bfloat16, "f32r": mybir.dt.float32r,
         "f32": mybir.dt.float32}
    return m[sim], m[e]


def _np_of(dt):
    from concourse import mybir
    if dt == mybir.dt.bfloat16:
        import ml_dtypes
        return ml_dtypes.bfloat16
    return np.float32


def _build_nc(reps=1, loop=1):
    import concourse.bass as bass
    import concourse.tile as tile
    from concourse import bacc, mybir

    F32 = mybir.dt.float32
    F32R = mybir.dt.float32r
    SIMDT, EDT = _dts()

    nc = bacc.Bacc(
        "TRN2", target_bir_lowering=False, debug=False, num_devices=8
    )
    xT = nc.dram_tensor("xT", [DIM, N], F32R, kind="ExternalInput").ap()
    wq = nc.dram_tensor("wq", [DIM, M], F32R, kind="ExternalInput").ap()
    wk = nc.dram_tensor("wk", [DIM, M], F32R, kind="ExternalInput").ap()
    wv = nc.dram_tensor("wv", [DIM, M], F32R, kind="ExternalInput").ap()
    wo = nc.dram_tensor("wo", [M, DIM], F32R, kind="ExternalInput").ap()
    ones_e = nc.dram_tensor("ones_e", [HD], EDT, kind="ExternalInput").ap()
    zeros_s = nc.dram_tensor("zeros_s", [P], SIMDT, kind="ExternalInput").ap()
    out = nc.dram_tensor("out", [N, DIM], F32, kind="ExternalOutput").ap()
    # DRAM bounce buffer for the reciprocal partition-broadcast
    rscratch = nc.dram_tensor("rscratch", [MH * NB * 2, 512], F32R).ap()

    with tile.TileContext(nc) as tc:
        from contextlib import ExitStack

        with nc.allow_low_precision(reason="f32r/bf16 matmul pipeline"), \
                ExitStack() as ctx:
            persist = ctx.enter_context(tc.tile_pool(name="persist", bufs=1))
            e_pool = ctx.enter_context(tc.tile_pool(name="e", bufs=6))
            r_pool = ctx.enter_context(tc.tile_pool(name="r", bufs=4))
            o_pool = ctx.enter_context(tc.tile_pool(name="o", bufs=4))
            # PSUM: pA = 4 x [128,512] (proj accums / ctx accums / rbcast),
            #       pB = 2 x [128,1024] (sim+exp), total exactly 8 banks
            pA = ctx.enter_context(tc.tile_pool(name="pA", bufs=4, space="PSUM"))
            pB = ctx.enter_context(tc.tile_pool(name="pB", bufs=2, space="PSUM"))

            env = {}
            env["xT_sb"] = persist.tile([P, KC, N], F32R, name="xT_sb")
            env["ctxT_sb"] = persist.tile([P, MC, N], F32R, name="ctxT_sb")
            env["kT_sb"] = persist.tile([P, MC, N], SIMDT, name="kT_sb")
            env["qdup"] = [persist.tile([P, N], SIMDT, name=f"qdup{h}") for h in range(MH)]
            env["kdiag"] = [persist.tile([P, NCH, P], SIMDT, name=f"kdiag{h}") for h in range(MH)]
            env["v_sb"] = persist.tile([P, MH, NCH, HD + 1], EDT, name="v_sb")
            env["wq_sb"] = persist.tile([P, KC, M], F32R, name="wq_sb")
            env["wk_sb"] = persist.tile([P, KC, M], F32R, name="wk_sb")
            env["wv_sb"] = persist.tile([P, KC, M], F32R, name="wv_sb")
            env["wo_sb"] = persist.tile([P, MC, DIM], F32R, name="wo_sb")
            nc.sync.dma_start(env["wq_sb"][:], wq.rearrange("(c p) m -> p c m", p=P))
            nc.sync.dma_start(env["wk_sb"][:], wk.rearrange("(c p) m -> p c m", p=P))
            nc.sync.dma_start(env["wv_sb"][:], wv.rearrange("(c p) m -> p c m", p=P))
            nc.sync.dma_start(env["wo_sb"][:], wo.rearrange("(c p) m -> p c m", p=P))
            # v_aug ones column
            for h in range(MH):
                nc.gpsimd.dma_start(
                    env["v_sb"][:, h, :, HD:HD + 1],
                    bass.AP(tensor=ones_e.tensor, offset=ones_e.offset,
                            ap=[[0, P], [0, NCH], [1, 1]]),
                )
            # kdiag off-diagonal zeros (never overwritten afterwards)
            for h in range(MH):
                for (r0, c0) in ((0, HD), (HD, 0)):
                    nc.gpsimd.dma_start(
                        env["kdiag"][h][r0:r0 + HD, :, c0:c0 + HD],
                        bass.AP(tensor=zeros_s.tensor, offset=zeros_s.offset,
                                ap=[[0, HD], [0, NCH], [1, HD]]),
                    )

            if os.environ.get("BASSK_PHASE", "all") == "attn":
                # timing probe: give the attention-phase tensors writers
                for h in range(MH):
                    nc.gpsimd.dma_start(
                        env["qdup"][h][:],
                        bass.AP(tensor=zeros_s.tensor, offset=zeros_s.offset,
                                ap=[[0, P], [0, N // P], [1, P]]),
                    )
                    nc.gpsimd.dma_start(
                        env["kdiag"][h][:],
                        bass.AP(tensor=zeros_s.tensor, offset=zeros_s.offset,
                                ap=[[0, P], [0, NCH], [1, P]]),
                    )
                    nc.gpsimd.dma_start(
                        env["v_sb"][:, h, :, 0:HD],
                        bass.AP(tensor=ones_e.tensor, offset=ones_e.offset,
                                ap=[[0, P], [0, NCH], [1, HD]]),
                    )

            env.update(xT=xT, out=out, F32=F32, F32R=F32R, SIMDT=SIMDT,
                       EDT=EDT, e_pool=e_pool, r_pool=r_pool, o_pool=o_pool,
                       pA=pA, pB=pB, rscratch=rscratch)

            if loop > 1:
                hint = ()
                if os.environ.get("BASSK_HINT", "1") == "1":
                    hint = (mybir.EngineType.PE, mybir.EngineType.Activation,
                            mybir.EngineType.DVE, mybir.EngineType.SP,
                            mybir.EngineType.Pool)
                with tc.For_i(0, loop, 1, hint_engines=hint):
                    _emit_rep(nc, tc, env)
            else:
                for rep in range(reps):
                    _emit_rep(nc, tc, env)

    nc.compile()
    return nc


def _emit_rep(nc, tc, env):
    from concourse import mybir

    EXP = mybir.ActivationFunctionType.Exp
    xT, out = env["xT"], env["out"]
    F32, F32R, SIMDT, EDT = env["F32"], env["F32R"], env["SIMDT"], env["EDT"]
    xT_sb, ctxT_sb, kT_sb = env["xT_sb"], env["ctxT_sb"], env["kT_sb"]
    qdup, kdiag, v_sb = env["qdup"], env["kdiag"], env["v_sb"]
    wq_sb, wk_sb, wv_sb, wo_sb = (env["wq_sb"], env["wk_sb"], env["wv_sb"],
                                  env["wo_sb"])
    e_pool, r_pool, o_pool = env["e_pool"], env["r_pool"], env["o_pool"]
    pA, pB = env["pA"], env["pB"]

    phase = os.environ.get("BASSK_PHASE", "all")

    for c in range(KC):
        nc.sync.dma_start(xT_sb[:, c, :], xT[c * P:(c + 1) * P, :])

    env["interleave_final"] = (
        phase == "all" and os.environ.get("BASSK_ILF", "0") == "1"
    )
    if phase in ("all", "proj"):
        _emit_proj(nc, tc, env)
    if phase in ("all", "attn"):
        _emit_attn(nc, tc, env)
    if phase == "all" and not env["interleave_final"]:
        _emit_final(nc, tc, env)


def _emit_proj(nc, tc, env):
    from concourse import mybir

    F32, F32R, SIMDT, EDT = env["F32"], env["F32R"], env["SIMDT"], env["EDT"]
    xT_sb, ctxT_sb, kT_sb = env["xT_sb"], env["ctxT_sb"], env["kT_sb"]
    qdup, kdiag, v_sb = env["qdup"], env["kdiag"], env["v_sb"]
    wq_sb, wk_sb, wv_sb, wo_sb = (env["wq_sb"], env["wk_sb"], env["wv_sb"],
                                  env["wo_sb"])
    pA = env["pA"]

    # q/k projections: kc-outer so the stationary weight chunk stays loaded
    # across the 4 n-blocks (fixed-stationary f32r measured ~230ns vs 336).
    for t, wsb in ((0, wq_sb), (1, wk_sb)):
        for mc in range(MC):
            accs = [pA.tile([P, 512], F32, tag="a", name=f"acc{i}") for i in range(4)]
            for kc in range(KC):
                for nb in range(4):
                    nc.tensor.matmul(
                        accs[nb][:],
                        wsb[:, kc, mc * P:(mc + 1) * P],
                        xT_sb[:, kc, nb * 512:(nb + 1) * 512],
                        start=(kc == 0),
                        stop=(kc == KC - 1),
                    )
            he, ho = 2 * mc, 2 * mc + 1
            for nb in range(4):
                ns = slice(nb * 512, (nb + 1) * 512)
                if t == 0:
                    # q: straight into the matching halves of qdup
                    nc.vector.tensor_copy(qdup[he][0:HD, ns], accs[nb][0:HD, :])
                    nc.vector.tensor_copy(qdup[ho][HD:P, ns], accs[nb][HD:P, :])
                else:
                    nc.vector.tensor_copy(kT_sb[:, mc, ns], accs[nb][:])
            if t == 0:
                # duplicate each head's rows onto the other partition half
                nc.sync.dma_start(qdup[he][HD:P, :], qdup[he][0:HD, :])
                nc.sync.dma_start(qdup[ho][0:HD, :], qdup[ho][HD:P, :])
            else:
                # scatter kT into the block-diagonal stationaries
                for h in (he, ho):
                    po = (h % 2) * HD
                    src = kT_sb[po:po + HD, mc, :].rearrange(
                        "p (j two d) -> p j two d", two=2, d=HD)
                    nc.sync.dma_start(kdiag[h][0:HD, :, 0:HD], src[:, :, 0, :])
                    nc.sync.dma_start(kdiag[h][HD:P, :, HD:P], src[:, :, 1, :])

    # v projection into per-head ones-augmented tiles
    for jc in range(NCH):
        ps = pA.tile([P, 512], F32, tag="a")
        for kc in range(KC):
            nc.tensor.matmul(
                ps[:, :M],
                xT_sb[:, kc, jc * P:(jc + 1) * P],
                wv_sb[:, kc, :],
                start=(kc == 0),
                stop=(kc == KC - 1),
            )
        for h in range(MH):
            nc.vector.tensor_copy(
                v_sb[:, h, jc, 0:HD], ps[:, h * HD:(h + 1) * HD]
            )


def _emit_attn(nc, tc, env):
    from concourse import mybir

    EXP = mybir.ActivationFunctionType.Exp
    F32, F32R, SIMDT, EDT = env["F32"], env["F32R"], env["SIMDT"], env["EDT"]
    ctxT_sb = env["ctxT_sb"]
    qdup, kdiag, v_sb = env["qdup"], env["kdiag"], env["v_sb"]
    rscratch = env["rscratch"]
    e_pool, r_pool = env["e_pool"], env["r_pool"]
    pA, pB = env["pA"], env["pB"]

    # attention: software-pipelined sim -> exp -> PV per (i-block, head)
    pending_norm = [None]

    def emit_unit(ib, h):
        po = (h % 2) * HD
        mc = h // 2
        i0 = ib * IB
        ctxs = [None, None]
        sims = {}
        ets = {}
        # PV lags the sim by 2 steps so exp(j) is already finished when the
        # in-order PE stream reaches PV(j) — otherwise PE idles ~exp-sim time
        # on every step.
        for jc in range(NCH + 2):
            if jc < NCH:
                sp = pB.tile([P, IB], F32, tag="b")
                for half in range(2):
                    nc.tensor.matmul(
                        sp[:, half * 512:(half + 1) * 512],
                        kdiag[h][:, jc, :],
                        qdup[h][:, i0 + half * 512:i0 + (half + 1) * 512],
                        start=True,
                        stop=True,
                    )
                sims[jc] = sp
            je = jc - 1
            if 0 <= je < NCH:
                sp = sims.pop(je)
                et = e_pool.tile([P, IB], EDT, tag="e")
                nc.scalar.activation(et[:], sp[:], EXP)
                ets[je] = et
            j = jc - 2
            if j < 0:
                continue
            et = ets.pop(j)
            if j == 0:
                if pending_norm[0] is not None:
                    pending_norm[0]()
                    pending_norm[0] = None
                ctxs[0] = pA.tile([HD + 1, 512], F32, tag="a", name="ctxA")
                ctxs[1] = pA.tile([HD + 1, 512], F32, tag="a", name="ctxB")
            for half in range(2):
                nc.tensor.matmul(
                    ctxs[half][:],
                    v_sb[:, h, j, :],
                    et[:, half * 512:(half + 1) * 512],
                    start=(j == 0),
                    stop=(j == NCH - 1),
                )

        def norm(ctxs=ctxs, po=po, mc=mc, i0=i0, ib=ib, h=h):
            # reciprocal of the denominator row, partition-broadcast via a
            # DRAM bounce (no PE, no extra PSUM; its latency hides in the
            # ~20us before these ctx slots are needed again)
            for half in range(2):
                cps = ctxs[half]
                isl = slice(i0 + half * 512, i0 + (half + 1) * 512)
                slot = (ib * MH + h) * 2 + half
                rc = r_pool.tile([1, 512], F32R, tag="rc")
                nc.vector.reciprocal(rc[:], cps[HD:HD + 1, :])
                nc.sync.dma_start(rscratch[slot:slot + 1, :], rc[:])
                rb = r_pool.tile([HD, 512], F32R, tag="rb")
                nc.sync.dma_start(
                    rb[:], rscratch[slot:slot + 1, :].to_broadcast((HD, 512))
                )
                nc.vector.tensor_mul(
                    ctxT_sb[po:po + HD, mc, isl], cps[0:HD, :], rb[:]
                )

        pending_norm[0] = norm

    for ib in range(NB):
        for h in range(MH):
            emit_unit(ib, h)
            if h == 0 and ib > 0 and env.get("interleave_final"):
                # previous i-block is fully normalized (its last pending norm
                # fired inside emit_unit above) — its output projection can
                # fill PE slack while this i-block's ACT-bound attention runs
                _emit_final_block(nc, env, ib - 1)
    pending_norm[0]()
    if env.get("interleave_final"):
        _emit_final_block(nc, env, NB - 1)


def _emit_final_block(nc, env, ib):
    F32 = env["F32"]
    ctxT_sb, wo_sb = env["ctxT_sb"], env["wo_sb"]
    o_pool, pA = env["o_pool"], env["pA"]
    out = env["out"]
    for nck in range(ib * (NCH // NB), (ib + 1) * (NCH // NB)):
        ps = pA.tile([P, 512], F32, tag="a", name="fps")
        for mc in range(MC):
            nc.tensor.matmul(
                ps[:],
                ctxT_sb[:, mc, nck * P:(nck + 1) * P],
                wo_sb[:, mc, :],
                start=(mc == 0),
                stop=(mc == MC - 1),
            )
        ot = o_pool.tile([P, 512], F32, tag="ot")
        nc.vector.tensor_copy(ot[:], ps[:])
        nc.sync.dma_start(out[nck * P:(nck + 1) * P, :], ot[:])


def _emit_final(nc, tc, env):
    F32, F32R = env["F32"], env["F32R"]
    ctxT_sb, wo_sb = env["ctxT_sb"], env["wo_sb"]
    o_pool, pA = env["o_pool"], env["pA"]
    out = env["out"]

    # output projection part = ctx @ Wo_s
    for nck in range(NCH):
        ps = pA.tile([P, 512], F32, tag="a")
        for mc in range(MC):
            nc.tensor.matmul(
                ps[:],
                ctxT_sb[:, mc, nck * P:(nck + 1) * P],
                wo_sb[:, mc, :],
                start=(mc == 0),
                stop=(mc == MC - 1),
            )
        ot = o_pool.tile([P, 512], F32, tag="ot")
        nc.vector.tensor_copy(ot[:], ps[:])
        nc.sync.dma_start(out[nck * P:(nck + 1) * P, :], ot[:])


def _get_nc(reps=1, loop=1):
    key = ("nc", reps, loop, os.environ.get("BASSK_SIMDT", "bf16"),
           os.environ.get("BASSK_EDT", "f32r"),
           os.environ.get("BASSK_PHASE", "all"))
    if key not in _CACHE:
        _CACHE[key] = _build_nc(reps, loop)
    return _CACHE[key]


def make_in_maps(x, Wq, Wkv, Wo):
    SIMDT, EDT = _dts()
    x = np.asarray(x, dtype=np.float32)
    Wq = np.asarray(Wq, dtype=np.float32)
    Wkv = np.asarray(Wkv, dtype=np.float32)
    Wo = np.asarray(Wo, dtype=np.float32)
    in_maps = []
    for c in range(8):
        b, hg = divmod(c, HG)
        sl = slice(hg * M, (hg + 1) * M)
        in_maps.append({
            "xT": np.ascontiguousarray(x[b].T),
            "wq": np.ascontiguousarray(Wq[:, sl]) * np.float32(SCALE),
            "wk": np.ascontiguousarray(Wkv[:, :DIM][:, sl]),
            "wv": np.ascontiguousarray(Wkv[:, DIM:][:, sl]),
            "wo": np.ascontiguousarray(Wo[sl, :]),
            "ones_e": np.ones(HD, dtype=_np_of(EDT)),
            "zeros_s": np.zeros(P, dtype=_np_of(SIMDT)),
        })
    return in_maps


def gather_out(results):
    out = np.zeros((B, N, DIM), dtype=np.float32)
    for c in range(8):
        out[c // HG] += results[c]["out"]
    return out


def kernel(x, Wq, Wkv, Wo):
    from concourse.bass_utils import run_bass_kernel_spmd

    nc = _get_nc()
    in_maps = make_in_maps(x, Wq, Wkv, Wo)
    res = run_bass_kernel_spmd(nc, in_maps, core_ids=list(range(8)))
    return gather_out(res.results)



# revision 2
# speedup vs baseline: 1.0265x; 1.0265x over previous
# Multi-head attention block (B=4, N=2048, DIM=512, H=8, HD=64) on 8 TRN2
# cores — transfer-minimized version.
#
# Sharding: batch x sequence. Core c handles batch b = c//2 and query half
# s = c%2 (rows s*1024..s*1024+1023), ALL 8 heads. K/V need every position,
# so the two cores of a batch pair-AllGather their x halves on device. The
# output rows per core are disjoint -> no output collective, no host summing.
#
# Per-call host<->device traffic is only x in (bf16, 1MB/core) and out
# (bf16, 1MB/core). The weights are baked into the NEFF as inline Const
# tensors (uploaded once at model load), so repeat calls move 16MB total
# instead of the 117MB the ExternalInput + f32 + zero-donation path moved.
#
# Device dataflow (activations transposed to [feature, position] via PE
# transposes; bf16 matmul pipeline):
#   xh    [1024, 512]  own half, natural layout (ExternalInput, bf16)
#   xfull [2048, 512]  pair AllGather of xh (internal DRAM)
#   xT_own [128,4,1024] PE-transposed xh     -> Q projection
#   xT_sb  [128,4,2048] PE-transposed xfull  -> K/V projections
#   qdup_h [128,1024]  q rows of head h duplicated on both partition halves
#   kdiag_h [128,16,128] block-diagonal stationary: diag(kT_h[jcA], kT_h[jcB])
#                      so the QK^T matmul runs with K=128 (K=64 is slower)
#   v_aug  [j, 65] per head, col 64 = 1.0: PV row 64 = softmax denominator
#   simT  [j,i] -> exp on ACT ([128,1024] to amortize ACTIVATE startup)
#   ctxT normalized by a DMA-bounce-broadcast reciprocal of row 64, then
#   out_half [1024, 512] = ctxT^T-stationary @ Wo (complete rows, bf16 out).
#
# The attention loop is software-pipelined BY EMISSION ORDER: sim for step
# j+1 is emitted before PV of step j so the in-order PE stream never stalls
# on ACT's exp.
import hashlib
import os

import numpy as np
import ml_dtypes

BF16NP = ml_dtypes.bfloat16

B, N, DIM = 4, 2048, 512
HEADS, HD = 8, 64
P = 128
NH = N // 2                 # own query rows per core = 1024
KC = DIM // P               # 4 contraction chunks
IB = NH                     # exp/i-block width = all own queries
NCH = N // P                # 16 j chunks
QNB = NH // 512             # 2 moving blocks for Q proj
KNB = N // 512              # 4 moving blocks for K proj
SCALE = HD ** -0.5
PAIRS = [[0, 1], [2, 3], [4, 5], [6, 7]]

_CACHE: dict = {}


def _rearr(w):
    # [DIM, M] -> [P, KC, M] with k = kc*P + p
    return np.ascontiguousarray(
        w.reshape(KC, P, w.shape[1]).transpose(1, 0, 2))


def _build_nc(Wq, Wkv, Wo, reps=1, loop=1):
    import concourse.bass as bass
    import concourse.tile as tile
    from concourse import bacc, mybir

    F32 = mybir.dt.float32
    F32R = mybir.dt.float32r
    BF = mybir.dt.bfloat16
    # exp/PV dtype: bf16 — full-rate PV matmul, 2x ACT exp throughput, and
    # memset accepts a bf16 set value (f32r is rejected by the ISA checker)
    EDT = BF

    nc = bacc.Bacc(
        "TRN2", target_bir_lowering=False, debug=False, num_devices=8
    )
    xh = nc.dram_tensor("xh", [NH, DIM], BF, kind="ExternalInput").ap()
    outh = nc.dram_tensor("outh", [NH, DIM], BF, kind="ExternalOutput").ap()
    bounce = nc.dram_tensor("bounce", [NH, DIM], BF).ap()
    xfull = nc.dram_tensor("xfull", [N, DIM], BF).ap()
    rscratch = nc.dram_tensor("rscratch", [HEADS * 2, 512], F32R).ap()

    wq_c = nc.inline_tensor(_rearr(Wq * SCALE).astype(BF16NP), "wq_c").ap()
    wk_c = nc.inline_tensor(_rearr(Wkv[:, :DIM]).astype(BF16NP), "wk_c").ap()
    wv_c = nc.inline_tensor(_rearr(Wkv[:, DIM:]).astype(BF16NP), "wv_c").ap()
    wo_c = nc.inline_tensor(_rearr(Wo).astype(BF16NP), "wo_c").ap()
    id_c = nc.inline_tensor(np.eye(P, dtype=BF16NP), "id_c").ap()

    with tile.TileContext(nc) as tc:
        from contextlib import ExitStack

        with nc.allow_low_precision(reason="bf16 matmul pipeline"), \
                ExitStack() as ctx:
            persist = ctx.enter_context(tc.tile_pool(name="persist", bufs=1))
            e_pool = ctx.enter_context(tc.tile_pool(name="e", bufs=5))
            r_pool = ctx.enter_context(tc.tile_pool(name="r", bufs=4))
            o_pool = ctx.enter_context(tc.tile_pool(name="o", bufs=4))
            x_pool = ctx.enter_context(tc.tile_pool(name="x", bufs=4))
            # PSUM: pA = 4 x [128,512] f32 banks, pB = 2 x [128,1024] f32
            pA = ctx.enter_context(tc.tile_pool(name="pA", bufs=4, space="PSUM"))
            pB = ctx.enter_context(tc.tile_pool(name="pB", bufs=2, space="PSUM"))

            env = {}
            env["xT_own"] = persist.tile([P, KC, NH], BF, name="xT_own")
            env["xT_sb"] = persist.tile([P, KC, N], BF, name="xT_sb")
            env["kT_sb"] = persist.tile([P, KC, N], BF, name="kT_sb")
            env["ctxT_sb"] = persist.tile([P, KC, NH], BF, name="ctxT_sb")
            env["qdup"] = [persist.tile([P, NH], BF, name=f"qdup{h}")
                           for h in range(HEADS)]
            env["kdiag"] = [persist.tile([P, NCH, P], BF, name=f"kdiag{h}")
                            for h in range(HEADS)]
            env["v_sb"] = persist.tile([P, HEADS, NCH, HD + 1], EDT, name="v_sb")
            env["wq_sb"] = persist.tile([P, KC, DIM], BF, name="wq_sb")
            env["wk_sb"] = persist.tile([P, KC, DIM], BF, name="wk_sb")
            env["wv_sb"] = persist.tile([P, KC, DIM], BF, name="wv_sb")
            env["wo_sb"] = persist.tile([P, KC, DIM], BF, name="wo_sb")
            env["id_sb"] = persist.tile([P, P], BF, name="id_sb")

            nc.sync.dma_start(env["wq_sb"][:], wq_c)
            nc.sync.dma_start(env["wk_sb"][:], wk_c)
            nc.sync.dma_start(env["wv_sb"][:], wv_c)
            nc.sync.dma_start(env["wo_sb"][:], wo_c)
            nc.sync.dma_start(env["id_sb"][:], id_c)
            # v_aug ones column: preset the whole tile to 1.0 (cols 0:HD are
            # overwritten by the V projection every rep; col HD stays 1.0)
            nc.gpsimd.memset(env["v_sb"][:], 1.0)
            # kdiag off-diagonal zeros: preset whole tiles (the K scatter
            # rewrites only the diagonal blocks every rep)
            for h in range(HEADS):
                nc.gpsimd.memset(env["kdiag"][h][:], 0.0)

            env.update(xh=xh, outh=outh, bounce=bounce, xfull=xfull,
                       rscratch=rscratch, F32=F32, F32R=F32R, BF=BF, EDT=EDT,
                       e_pool=e_pool, r_pool=r_pool, o_pool=o_pool,
                       x_pool=x_pool, pA=pA, pB=pB, mybir=mybir)

            if loop > 1:
                # collectives deadlock inside a hardware loop (NRT), so the
                # timing NEFF hoists the gather out; the loop body times
                # everything else (the AllGather is a ~constant adder)
                _emit_gather(nc, env)
                hint = (mybir.EngineType.PE, mybir.EngineType.Activation,
                        mybir.EngineType.DVE, mybir.EngineType.SP,
                        mybir.EngineType.Pool)
                with tc.For_i(0, loop, 1, hint_engines=hint):
                    _emit_rep(nc, tc, env, skip_gather=True)
            else:
                for _ in range(reps):
                    _emit_rep(nc, tc, env)

    nc.compile()
    return nc


def _emit_gather(nc, env):
    mybir = env["mybir"]
    xh, bounce, xfull = env["xh"], env["bounce"], env["xfull"]
    # stage own half into internal DRAM, pair-AllGather to the full batch
    nc.sync.dma_start(bounce, xh)
    nc.gpsimd.collective_compute(
        "AllGather", mybir.AluOpType.bypass,
        replica_groups=PAIRS,
        ins=[bounce.opt()], outs=[xfull.opt()],
    )


def _emit_rep(nc, tc, env, skip_gather=False):
    xh, xfull = env["xh"], env["xfull"]

    if not skip_gather:
        _emit_gather(nc, env)

    # transpose own half -> xT_own (overlaps the AllGather), then Q proj
    _emit_transpose(nc, env, xh, env["xT_own"], NH)
    _emit_qproj(nc, env)
    # transpose gathered x -> xT_sb, then K/V projections
    _emit_transpose(nc, env, xfull, env["xT_sb"], N)
    _emit_kvproj(nc, env)
    _emit_attn(nc, env)
    _emit_final(nc, env)


def _emit_transpose(nc, env, src_dram, dst_sb, n):
    BF = env["BF"]
    x_pool, pA = env["x_pool"], env["pA"]
    id_sb = env["id_sb"]
    for t in range(n // P):
        xt = x_pool.tile([P, DIM], BF, tag="xt")
        nc.sync.dma_start(xt[:], src_dram[t * P:(t + 1) * P, :])
        for kc in range(KC):
            pt = pA.tile([P, P], BF, tag="a", name="ptr")
            nc.tensor.transpose(pt[:], xt[:, kc * P:(kc + 1) * P], id_sb[:])
            nc.vector.tensor_copy(dst_sb[:, kc, t * P:(t + 1) * P], pt[:])


def _emit_qproj(nc, env):
    F32 = env["F32"]
    xT_own, wq_sb, qdup = env["xT_own"], env["wq_sb"], env["qdup"]
    pA = env["pA"]

    for mc in range(KC):
        accs = [pA.tile([P, 512], F32, tag="a", name=f"qacc{i}")
                for i in range(QNB)]
        for kc in range(KC):
            for nb in range(QNB):
                nc.tensor.matmul(
                    accs[nb][:],
                    wq_sb[:, kc, mc * P:(mc + 1) * P],
                    xT_own[:, kc, nb * 512:(nb + 1) * 512],
                    start=(kc == 0),
                    stop=(kc == KC - 1),
                )
        he, ho = 2 * mc, 2 * mc + 1
        for nb in range(QNB):
            ns = slice(nb * 512, (nb + 1) * 512)
            nc.vector.tensor_copy(qdup[he][0:HD, ns], accs[nb][0:HD, :])
            nc.vector.tensor_copy(qdup[ho][HD:P, ns], accs[nb][HD:P, :])
        # duplicate each head's rows onto the other partition half
        nc.sync.dma_start(qdup[he][HD:P, :], qdup[he][0:HD, :])
        nc.sync.dma_start(qdup[ho][0:HD, :], qdup[ho][HD:P, :])


def _emit_kvproj(nc, env):
    F32 = env["F32"]
    xT_sb, kT_sb = env["xT_sb"], env["kT_sb"]
    wk_sb, wv_sb = env["wk_sb"], env["wv_sb"]
    kdiag, v_sb = env["kdiag"], env["v_sb"]
    pA = env["pA"]

    # K projection over the full gathered sequence
    for mc in range(KC):
        accs = [pA.tile([P, 512], F32, tag="a", name=f"kacc{i}")
                for i in range(KNB)]
        for kc in range(KC):
            for nb in range(KNB):
                nc.tensor.matmul(
                    accs[nb][:],
                    wk_sb[:, kc, mc * P:(mc + 1) * P],
                    xT_sb[:, kc, nb * 512:(nb + 1) * 512],
                    start=(kc == 0),
                    stop=(kc == KC - 1),
                )
        for nb in range(KNB):
            ns = slice(nb * 512, (nb + 1) * 512)
            nc.vector.tensor_copy(kT_sb[:, mc, ns], accs[nb][:])
        # scatter kT into the block-diagonal stationaries
        for h in (2 * mc, 2 * mc + 1):
            po = (h % 2) * HD
            src = kT_sb[po:po + HD, mc, :].rearrange(
                "p (j two d) -> p j two d", two=2, d=HD)
            nc.sync.dma_start(kdiag[h][0:HD, :, 0:HD], src[:, :, 0, :])
            nc.sync.dma_start(kdiag[h][HD:P, :, HD:P], src[:, :, 1, :])

    # V projection into per-head ones-augmented tiles
    for jc in range(NCH):
        ps = pA.tile([P, 512], F32, tag="a")
        for kc in range(KC):
            nc.tensor.matmul(
                ps[:],
                xT_sb[:, kc, jc * P:(jc + 1) * P],
                wv_sb[:, kc, :],
                start=(kc == 0),
                stop=(kc == KC - 1),
            )
        for h in range(HEADS):
            nc.vector.tensor_copy(
                v_sb[:, h, jc, 0:HD], ps[:, h * HD:(h + 1) * HD]
            )


def _emit_attn(nc, env):
    mybir = env["mybir"]
    EXP = mybir.ActivationFunctionType.Exp
    F32, F32R, EDT = env["F32"], env["F32R"], env["EDT"]
    ctxT_sb = env["ctxT_sb"]
    qdup, kdiag, v_sb = env["qdup"], env["kdiag"], env["v_sb"]
    rscratch = env["rscratch"]
    e_pool, r_pool = env["e_pool"], env["r_pool"]
    pA, pB = env["pA"], env["pB"]

    pending_norm = [None]

    def emit_unit(h):
        po = (h % 2) * HD
        mc = h // 2
        ctxs = [None, None]
        sims = {}
        ets = {}
        # PV lags sim by 2 steps so exp(j) is finished when the in-order PE
        # stream reaches PV(j)
        for jc in range(NCH + 2):
            if jc < NCH:
                sp = pB.tile([P, IB], F32, tag="b")
                for half in range(2):
                    nc.tensor.matmul(
                        sp[:, half * 512:(half + 1) * 512],
                        kdiag[h][:, jc, :],
                        qdup[h][:, half * 512:(half + 1) * 512],
                        start=True,
                        stop=True,
                    )
                sims[jc] = sp
            je = jc - 1
            if 0 <= je < NCH:
                sp = sims.pop(je)
                et = e_pool.tile([P, IB], EDT, tag="e")
                nc.scalar.activation(et[:], sp[:], EXP)
                ets[je] = et
            j = jc - 2
            if j < 0:
                continue
            et = ets.pop(j)
            if j == 0:
                if pending_norm[0] is not None:
                    pending_norm[0]()
                    pending_norm[0] = None
                ctxs[0] = pA.tile([HD + 1, 512], F32, tag="a", name="ctxA")
                ctxs[1] = pA.tile([HD + 1, 512], F32, tag="a", name="ctxB")
            for half in range(2):
                nc.tensor.matmul(
                    ctxs[half][:],
                    v_sb[:, h, j, :],
                    et[:, half * 512:(half + 1) * 512],
                    start=(j == 0),
                    stop=(j == NCH - 1),
                )

        def norm(ctxs=ctxs, po=po, mc=mc, h=h):
            # reciprocal of the denominator row, partition-broadcast via a
            # DRAM bounce (no PE, no extra PSUM; latency hides before these
            # ctx slots are needed again)
            for half in range(2):
                cps = ctxs[half]
                isl = slice(half * 512, (half + 1) * 512)
                slot = h * 2 + half
                rc = r_pool.tile([1, 512], F32R, tag="rc")
                nc.vector.reciprocal(rc[:], cps[HD:HD + 1, :])
                nc.sync.dma_start(rscratch[slot:slot + 1, :], rc[:])
                rb = r_pool.tile([HD, 512], F32R, tag="rb")
                nc.sync.dma_start(
                    rb[:], rscratch[slot:slot + 1, :].to_broadcast((HD, 512))
                )
                nc.vector.tensor_mul(
                    ctxT_sb[po:po + HD, mc, isl], cps[0:HD, :], rb[:]
                )

        pending_norm[0] = norm

    for h in range(HEADS):
        emit_unit(h)
    pending_norm[0]()


def _emit_final(nc, env):
    F32, BF = env["F32"], env["BF"]
    ctxT_sb, wo_sb = env["ctxT_sb"], env["wo_sb"]
    o_pool, pA = env["o_pool"], env["pA"]
    outh = env["outh"]

    for nck in range(NH // P):
        ps = pA.tile([P, 512], F32, tag="a")
        for mc in range(KC):
            nc.tensor.matmul(
                ps[:],
                ctxT_sb[:, mc, nck * P:(nck + 1) * P],
                wo_sb[:, mc, :],
                start=(mc == 0),
                stop=(mc == KC - 1),
            )
        ot = o_pool.tile([P, 512], BF, tag="ot")
        nc.vector.tensor_copy(ot[:], ps[:])
        nc.sync.dma_start(outh[nck * P:(nck + 1) * P, :], ot[:])


def _wdigest(Wq, Wkv, Wo):
    h = hashlib.blake2b(digest_size=16)
    for w in (Wq, Wkv, Wo):
        a = np.ascontiguousarray(w, dtype=np.float32)
        h.update(str(a.shape).encode())
        h.update(a[::7].tobytes())
        h.update(a[1::13, ::3].tobytes())
    return h.hexdigest()


def _get_nc(Wq, Wkv, Wo, reps=1, loop=1):
    key = ("nc", _wdigest(Wq, Wkv, Wo), reps, loop)
    if key not in _CACHE:
        _CACHE[key] = _build_nc(np.asarray(Wq, np.float32),
                                np.asarray(Wkv, np.float32),
                                np.asarray(Wo, np.float32), reps, loop)
    return _CACHE[key]


def make_in_maps(x, Wq, Wkv, Wo):
    x = np.asarray(x, np.float32).reshape(8, NH, DIM).astype(BF16NP)
    return [{"xh": x[c]} for c in range(8)]


def gather_out(results):
    out = np.stack([r["outh"] for r in results])
    return out.astype(np.float32).reshape(B, N, DIM)


class _Runner:
    """Cached jitted executor: one compile, then warm calls move only
    x in (bf16) and out (bf16); weights live in the NEFF as consts."""

    def __init__(self, nc):
        import jax
        from jax.sharding import Mesh, PartitionSpec, NamedSharding
        from jax.experimental.shard_map import shard_map
        from concourse import mybir
        from concourse.bass2jax import (
            _bass_exec_p, install_neuronx_cc_hook, partition_id_tensor)

        install_neuronx_cc_hook()
        self.jax = jax
        partition_name = (nc.partition_id_tensor.name
                          if nc.partition_id_tensor else None)
        in_names, out_names, out_avals = [], [], []
        for alloc in nc.m.functions[0].allocations:
            if not hasattr(alloc, "kind"):
                continue
            if not isinstance(alloc, mybir.MemoryLocationSet):
                continue
            name = alloc.memorylocations[0].name
            if alloc.kind == "ExternalInput":
                if name != partition_name:
                    in_names.append(name)
            elif alloc.kind == "ExternalOutput":
                out_names.append(name)
                out_avals.append(jax.core.ShapedArray(
                    tuple(alloc.tensor_shape), mybir.dt.np(alloc.dtype)))
        assert in_names == ["xh"] and out_names == ["outh"], (in_names, out_names)
        n_params = len(in_names)
        all_in = list(in_names) + list(out_names)
        if partition_name is not None:
            all_in.append(partition_name)

        def _body(*args):
            operands = list(args)
            if partition_name is not None:
                operands.append(partition_id_tensor())
            return tuple(_bass_exec_p.bind(
                *operands,
                out_avals=tuple(out_avals),
                in_names=tuple(all_in),
                out_names=tuple(out_names),
                lowering_input_output_aliases=(),
                sim_require_finite=True,
                sim_require_nnan=True,
                nc=nc,
            ))

        devices = jax.devices()[:8]
        mesh = Mesh(np.asarray(devices), ("core",))
        self.sharded = jax.jit(
            shard_map(_body, mesh=mesh,
                      in_specs=(PartitionSpec("core"),) * (n_params + 1),
                      out_specs=(PartitionSpec("core"),),
                      check_rep=False),
            keep_unused=True,
        )
        sh = NamedSharding(mesh, PartitionSpec("core"))
        # persistent device-resident dummy for the out operand (uploaded once)
        self.dummy_out = jax.device_put(
            np.zeros((8 * NH, DIM), BF16NP), sh)
        self.in_sharding = sh

    def __call__(self, x):
        xb = np.asarray(x, np.float32).reshape(8 * NH, DIM).astype(BF16NP)
        (out,) = self.sharded(xb, self.dummy_out)
        return np.asarray(out).astype(np.float32).reshape(B, N, DIM)


def kernel(x, Wq, Wkv, Wo):
    key = ("runner", _wdigest(Wq, Wkv, Wo))
    if key not in _CACHE:
        _CACHE[key] = _Runner(_get_nc(Wq, Wkv, Wo))
    return _CACHE[key](x)
